# revision 19
# baseline (speedup 1.0000x reference)
"""Trainium2 Bass kernel for CalibConv (5x5 conv -> linear -> drift modulation).

Math: the reference computes, per kernel position p=(i,j) and class k:
    cmap[n,h,w,p,k] = sum_c x_pad[n,c,h+i,w+j] * Weff[k,c,p] + beff[k]
where Weff[k,c,p] = sum_o Wlin[k,o]*Wt[o,c,i,j] and beff = Wlin@bias + blin
(the O=64 conv channels are linearly projected to 2 classes, so they fold
into the weights on the host). Then per output pixel:
    asum = sum_p |cmap|, ysum = sum_p yofs[p]*|cmap|, xsum likewise,
    csum = sum_p cmap,  out = csum * exp(-0.5*sqrt(ysum^2+xsum^2)/asum)

Device strategy (per core; data-parallel over batch N=8 across 8 cores):
  1. x arrives host-padded [C, 60*60]; DMA to SBUF (queues interleaved)
  2. one fp32r matmul pass per column chunk: weff has FOUR weight columns
     per kernel position p, at rows 4p+2+u (u=0,1: G' for class k=u;
     u=2,3: the same weights again for the |.| branch) -> psum [128, F].
     A single ACT Prelu per chunk applies bias and computes BOTH branches:
     alpha=+1 rows pass through (G'+beff), alpha=-1 rows give |G'+beff|.
  3. gather: the window shift for p=(i,j) in 60-stride padded layout is a
     pure flat offset 60*i+j, so each gather is one contiguous 4-partition
     SBUF->SBUF copy ag[4p+2:4p+6, off:off+L] -> bc[4p+2:4p+6, 0:L].
     Rows straddle two SBUF port groups (base 4p+2) for 2x DMA bandwidth.
  4. stats: per 2-padded-row chunk (120 positions x 28), matmul with
     stationary bc[:, off:off+120] [128 x 120] and moving smat [128, 8]
     -> PSUM [120 positions, 8 stats]; horizontal-halo rows are junk,
     skipped by the output DMA.
  5. pixel-major epilogue; 4 small output DMAs [3136, 2]
"""

import numpy as np

import concourse.bacc as bacc
import concourse.mybir as mybir
from concourse import tile
from concourse.bass_utils import run_bass_kernel_spmd

N_CORES = 8
C, H, W = 128, 56, 56
KS, PAD = 5, 2
HP, WP = H + 2 * PAD, W + 2 * PAD  # 60, 60
NPIX = HP * WP                      # 3600
NOUT = H * W                        # 3136
P25 = KS * KS                       # 25
NCLS = 2
MP = 128                            # row space: 4p+2+u (rows 0,1,102.. junk)
GLEN = (H - 1) * WP + W             # 3356: gather copy length per row

G_CHUNK = 450
N_GCHUNK = NPIX // G_CHUNK          # 8
S_CHUNK = 2 * WP                    # 120: stats stationary = 2 padded rows
N_SCHUNK = H // 2                   # 28 chunks (h rows 2c, 2c+1)
S_OFFS = [S_CHUNK * c for c in range(N_SCHUNK - 1)] + [GLEN - S_CHUNK]
S_SHIFT_LAST = S_CHUNK * (N_SCHUNK - 1) - (GLEN - S_CHUNK)  # 4

F32 = mybir.dt.float32
F32R = mybir.dt.float32r
AF = mybir.ActivationFunctionType


def row_base(p):
    return 4 * p + 2


BCSH = 8  # bc rows sit +8 partitions above ag rows: gather reads and writes
          # then hit different SBUF port groups (ports cycle every 4 rows)


def kernel_body(tc, x_d, weff_d, actp_d, smat_d, out_d):
    nc = tc.nc
    with (
        tc.tile_pool(name="const", bufs=1) as cpool,
        tc.tile_pool(name="big", bufs=1) as bpool,
        tc.tile_pool(name="psg", bufs=4, space="PSUM") as psg_pool,
        tc.tile_pool(name="pss", bufs=1, space="PSUM") as pss_pool,
        tc.tile_pool(name="tmp", bufs=1) as tpool,
    ):
        # warm the ACT Prelu table before the G phase
        warm_in = cpool.tile([1, 1], F32)
        warm_out = cpool.tile([1, 1], F32)
        nc.vector.memset(warm_in[:], 1.0)
        nc.scalar.activation(warm_out[:], warm_in[:], AF.Prelu, alpha=0.5)

        weff_sb = cpool.tile([C, MP], F32R)
        actp_sb = cpool.tile([MP, 2], F32)
        smat_sb = cpool.tile([MP, 8], F32R)
        nc.gpsimd.dma_start(weff_sb[:], weff_d[:])
        nc.gpsimd.dma_start(actp_sb[:], actp_d[:])
        nc.gpsimd.dma_start(smat_sb[:], smat_d[:])

        # input arrives pre-padded [C, 60*60]; interleave the two HWDGE queues
        xp = bpool.tile([C, NPIX], F32R)
        for b in range(N_GCHUNK):
            eng = nc.sync if b % 2 == 0 else nc.scalar
            eng.dma_start(
                xp[:, G_CHUNK * b : G_CHUNK * (b + 1)],
                x_d[:, G_CHUNK * b : G_CHUNK * (b + 1)],
            )

        # G phase: matmul + one Prelu per chunk writes all 100 live rows
        ag = bpool.tile([MP, NPIX], F32R)
        for g in range(N_GCHUNK):
            sl = slice(G_CHUNK * g, G_CHUNK * (g + 1))
            ps = psg_pool.tile([MP, G_CHUNK], F32, tag="psg")
            nc.tensor.matmul(ps[:], weff_sb[:], xp[:, sl], start=True, stop=True)
            nc.scalar.activation(
                ag[:, sl], ps[:], AF.Prelu,
                bias=actp_sb[:, 0:1], alpha=actp_sb[:, 1:2],
            )

        # ACT tables for the epilogue load now (engine order: after the G
        # Prelus, so the loads hide behind the gather phase)
        nc.scalar.activation(warm_out[:], warm_in[:], AF.Exp)
        nc.scalar.activation(warm_out[:], warm_in[:], AF.Sqrt)

        # gather: contiguous flat-offset copies (rows 4p+2..4p+5); first and
        # last extend to cover the junk rows so bc is fully initialized
        bc = bpool.tile([MP, NPIX], F32R)
        dma_engs = [nc.sync, nc.scalar, nc.gpsimd]
        HALF = 1680
        # junk-row fillers so bc is fully initialized (values unused: smat=0)
        nc.sync.dma_start(bc[0:BCSH, 0:GLEN], ag[0:BCSH, 0:GLEN])
        nc.scalar.dma_start(
            bc[MP - 18 : MP, 0:GLEN], ag[MP - 26 : MP - 8, 0:GLEN]
        )
        # all first halves before second halves: stats chunks in the first
        # half of the image unblock while the second half still gathers
        for h, (c0, c1) in enumerate(((0, HALF), (HALF, GLEN))):
            for p in range(P25):
                i, j = p // KS, p % KS
                off = WP * i + j
                r0 = 0 if p == 0 else row_base(p)
                r1 = MP - BCSH if p == P25 - 1 else row_base(p) + 4
                dma_engs[(2 * p + h) % 3].dma_start(
                    bc[r0 + BCSH : r1 + BCSH, c0:c1],
                    ag[r0:r1, off + c0 : off + c1],
                )

        # stats: out[pos, stat] = sum_rows bc[row, pos] * smat[row, stat]
        ps_stats = pss_pool.tile([S_CHUNK, N_SCHUNK, 8], F32)
        for s in range(N_SCHUNK):
            off = S_OFFS[s]
            nc.tensor.matmul(
                ps_stats[:, s, :],
                bc[:, off : off + S_CHUNK],
                smat_sb[:],
                start=True,
                stop=True,
            )

        # epilogue: out = csum * exp(-0.5 * sqrt(ysum^2 + xsum^2) / asum)
        # squares on DVE (via an SBUF copy) so ACT only needs Sqrt + Exp
        rinv = tpool.tile([S_CHUNK, N_SCHUNK, NCLS], F32)
        yx = tpool.tile([S_CHUNK, N_SCHUNK, 4], F32)
        yx2 = tpool.tile([S_CHUNK, N_SCHUNK, 4], F32)
        ssum = tpool.tile([S_CHUNK, N_SCHUNK, NCLS], F32)
        srt = tpool.tile([S_CHUNK, N_SCHUNK, NCLS], F32)
        drift = tpool.tile([S_CHUNK, N_SCHUNK, NCLS], F32)
        expd = tpool.tile([S_CHUNK, N_SCHUNK, NCLS], F32)
        outv = tpool.tile([S_CHUNK, N_SCHUNK, NCLS], F32)
        nc.vector.reciprocal(rinv[:], ps_stats[:, :, 0:2])
        nc.vector.tensor_copy(yx[:], ps_stats[:, :, 2:6])
        nc.vector.tensor_mul(yx2[:], yx[:], yx[:])
        nc.vector.tensor_add(ssum[:], yx2[:, :, 0:2], yx2[:, :, 2:4])
        nc.scalar.activation(srt[:], ssum[:], AF.Sqrt)
        nc.vector.tensor_mul(drift[:], srt[:], rinv[:])
        nc.scalar.activation(expd[:], drift[:], AF.Exp, scale=-0.5)
        nc.vector.tensor_mul(outv[:], ps_stats[:, :, 6:8], expd[:])

        # output: pixel (h=2c+d, w) lives at outv[60d+w, c] (chunks 0..26)
        # and outv[60d+4+w, 27] (shifted last chunk); halo partitions skipped
        out_dv = out_d[:].rearrange("(c d w) k -> d w c k", d=2, w=W)
        nlast = N_SCHUNK - 1
        for d in range(2):
            nc.sync.dma_start(
                out_dv[d, :, 0:nlast, :], outv[60 * d : 60 * d + W, 0:nlast, :]
            )
            nc.scalar.dma_start(
                out_dv[d, :, nlast, :],
                outv[60 * d + S_SHIFT_LAST : 60 * d + S_SHIFT_LAST + W, nlast, :],
            )


def build_program():
    nc = bacc.Bacc("TRN2", target_bir_lowering=False, debug=False)
    x_d = nc.dram_tensor("x", [C, NPIX], F32R, kind="ExternalInput").ap()
    weff_d = nc.dram_tensor("weff", [C, MP], F32R, kind="ExternalInput").ap()
    actp_d = nc.dram_tensor("actp", [MP, 2], F32, kind="ExternalInput").ap()
    smat_d = nc.dram_tensor("smat", [MP, 8], F32R, kind="ExternalInput").ap()
    out_d = nc.dram_tensor("out", [NOUT, NCLS], F32, kind="ExternalOutput").ap()
    with tile.TileContext(nc) as tc:
        kernel_body(tc, x_d, weff_d, actp_d, smat_d, out_d)
    nc.compile()
    return nc


def host_params(Wt, bias, Wlin, blin):
    """Fold conv weights + linear projection into device params."""
    Wt = np.asarray(Wt, np.float32)
    bias = np.asarray(bias, np.float32)
    Wlin = np.asarray(Wlin, np.float32)
    blin = np.asarray(blin, np.float32)
    O = Wt.shape[0]
    Wp = Wt.reshape(O, C, P25)                        # (O, C, P)
    Weff = np.einsum("ko,ocp->kcp", Wlin, Wp)         # (2, C, P)
    beff2 = (Wlin @ bias + blin).astype(np.float32)   # (2,)
    offs = np.arange(-PAD, PAD + 1, dtype=np.float32)

    wext = np.zeros((MP, C), np.float32)
    actp = np.zeros((MP, 2), np.float32)   # col0 = bias, col1 = prelu alpha
    actp[:, 1] = 1.0
    smat = np.zeros((MP, 8), np.float32)
    for p in range(P25):
        i, j = p // KS, p % KS
        for u in range(4):
            k = u % 2
            r = row_base(p) + u
            wext[r] = Weff[k, :, p]
            actp[r, 0] = beff2[k]
            actp[r, 1] = 1.0 if u < 2 else -1.0
            rb = r + BCSH                     # bc row (shifted vs ag row)
            if u < 2:
                smat[rb, 6 + k] = 1.0          # csum (G' rows)
            else:
                smat[rb, 0 + k] = 1.0          # asum
                smat[rb, 2 + k] = offs[i]      # ysum (yofs)
                smat[rb, 4 + k] = offs[j]      # xsum (xofs)
    weff = np.ascontiguousarray(wext.T)
    return weff, actp, smat


_nc_cache = None
last_results = None  # BassKernelResults of the most recent run (for profiling)


def kernel(x, Wt, bias, Wlin, blin):
    global _nc_cache, last_results
    x = np.asarray(x, np.float32)
    xpad = np.ascontiguousarray(
        np.pad(x, ((0, 0), (0, 0), (PAD, PAD), (PAD, PAD))).reshape(N_CORES, C, NPIX)
    )
    weff, actp, smat = host_params(Wt, bias, Wlin, blin)
    if _nc_cache is None:
        _nc_cache = build_program()
    in_maps = [
        {
            "x": xpad[n],
            "weff": weff,
            "actp": actp,
            "smat": smat,
        }
        for n in range(N_CORES)
    ]
    res = run_bass_kernel_spmd(_nc_cache, in_maps, list(range(N_CORES)))
    last_results = res
    out = np.stack(
        [res.results[n]["out"].reshape(H, W, NCLS) for n in range(N_CORES)]
    )
    return out


# revision 20
# speedup vs baseline: 1.0206x; 1.0206x over previous
"""Trainium2 Bass kernel for CalibConv (5x5 conv -> linear -> drift modulation).

Math: the reference computes, per kernel position p=(i,j) and class k:
    cmap[n,h,w,p,k] = sum_c x_pad[n,c,h+i,w+j] * Weff[k,c,p] + beff[k]
where Weff[k,c,p] = sum_o Wlin[k,o]*Wt[o,c,i,j] and beff = Wlin@bias + blin
(the O=64 conv channels are linearly projected to 2 classes, so they fold
into the weights on the host). Then per output pixel:
    asum = sum_p |cmap|, ysum = sum_p yofs[p]*|cmap|, xsum likewise,
    csum = sum_p cmap,  out = csum * exp(-0.5*sqrt(ysum^2+xsum^2)/asum)

Device strategy (per core; data-parallel over batch N=8 across 8 cores):
  1. x arrives host-padded [C, 60*60]; DMA to SBUF (queues interleaved)
  2. one fp32r matmul pass per column chunk: weff has FOUR weight columns
     per kernel position p, at rows 4p+2+u (u=0,1: G' for class k=u;
     u=2,3: the same weights again for the |.| branch) -> psum [128, F].
     A single ACT Prelu per chunk applies bias and computes BOTH branches:
     alpha=+1 rows pass through (G'+beff), alpha=-1 rows give |G'+beff|.
  3. gather: the window shift for p=(i,j) in 60-stride padded layout is a
     pure flat offset 60*i+j, so each gather is one contiguous 4-partition
     SBUF->SBUF copy ag[4p+2:4p+6, off:off+L] -> bc[4p+2:4p+6, 0:L].
     Rows straddle two SBUF port groups (base 4p+2) for 2x DMA bandwidth.
  4. stats: per 2-padded-row chunk (120 positions x 28), matmul with
     stationary bc[:, off:off+120] [128 x 120] and moving smat [128, 8]
     -> PSUM [120 positions, 8 stats]; horizontal-halo rows are junk,
     skipped by the output DMA.
  5. pixel-major epilogue; 4 small output DMAs [3136, 2]
"""

import numpy as np

import concourse.bacc as bacc
import concourse.mybir as mybir
from concourse import tile
from concourse.bass_utils import run_bass_kernel_spmd

N_CORES = 8
C, H, W = 128, 56, 56
KS, PAD = 5, 2
HP, WP = H + 2 * PAD, W + 2 * PAD  # 60, 60
NPIX = HP * WP                      # 3600
NOUT = H * W                        # 3136
P25 = KS * KS                       # 25
NCLS = 2
MP = 128                            # row space: 4p+2+u (rows 0,1,102.. junk)
GLEN = (H - 1) * WP + W             # 3356: gather copy length per row

G_CHUNK = 450
N_GCHUNK = NPIX // G_CHUNK          # 8
S_CHUNK = 2 * WP                    # 120: stats stationary = 2 padded rows
N_SCHUNK = H // 2                   # 28 chunks (h rows 2c, 2c+1)
S_OFFS = [S_CHUNK * c for c in range(N_SCHUNK - 1)] + [GLEN - S_CHUNK]
S_SHIFT_LAST = S_CHUNK * (N_SCHUNK - 1) - (GLEN - S_CHUNK)  # 4

F32 = mybir.dt.float32
F32R = mybir.dt.float32r
AF = mybir.ActivationFunctionType


def row_base(p):
    return 4 * p + 2


BCSH = 0  # bc rows aligned with ag rows


def kernel_body(tc, x_d, weff_d, actp_d, smat_d, out_d):
    nc = tc.nc
    with (
        tc.tile_pool(name="const", bufs=1) as cpool,
        tc.tile_pool(name="big", bufs=1) as bpool,
        tc.tile_pool(name="psg", bufs=4, space="PSUM") as psg_pool,
        tc.tile_pool(name="pss", bufs=1, space="PSUM") as pss_pool,
        tc.tile_pool(name="tmp", bufs=1) as tpool,
    ):
        # warm the ACT Prelu table before the G phase
        warm_in = cpool.tile([1, 1], F32)
        warm_out = cpool.tile([1, 1], F32)
        nc.vector.memset(warm_in[:], 1.0)
        nc.scalar.activation(warm_out[:], warm_in[:], AF.Prelu, alpha=0.5)

        weff_sb = cpool.tile([C, MP], F32R)
        actp_sb = cpool.tile([MP, 2], F32)
        smat_sb = cpool.tile([MP, 8], F32R)
        nc.gpsimd.dma_start(weff_sb[:], weff_d[:])
        nc.gpsimd.dma_start(actp_sb[:], actp_d[:])
        nc.gpsimd.dma_start(smat_sb[:], smat_d[:])

        # input arrives pre-padded [C, 60*60]; interleave the two HWDGE queues
        xp = bpool.tile([C, NPIX], F32R)
        for b in range(N_GCHUNK):
            eng = nc.sync if b % 2 == 0 else nc.scalar
            eng.dma_start(
                xp[:, G_CHUNK * b : G_CHUNK * (b + 1)],
                x_d[:, G_CHUNK * b : G_CHUNK * (b + 1)],
            )

        # G phase: matmul + one Prelu per chunk writes all 100 live rows
        ag = bpool.tile([MP, NPIX], F32R)
        for g in range(N_GCHUNK):
            sl = slice(G_CHUNK * g, G_CHUNK * (g + 1))
            ps = psg_pool.tile([MP, G_CHUNK], F32, tag="psg")
            nc.tensor.matmul(ps[:], weff_sb[:], xp[:, sl], start=True, stop=True)
            nc.scalar.activation(
                ag[:, sl], ps[:], AF.Prelu,
                bias=actp_sb[:, 0:1], alpha=actp_sb[:, 1:2],
            )

        # ACT tables for the epilogue load now (engine order: after the G
        # Prelus, so the loads hide behind the gather phase)
        nc.scalar.activation(warm_out[:], warm_in[:], AF.Exp)
        nc.scalar.activation(warm_out[:], warm_in[:], AF.Sqrt)

        # gather: contiguous flat-offset copies (rows 4p+2..4p+5); first and
        # last extend to cover the junk rows so bc is fully initialized
        bc = bpool.tile([MP, NPIX], F32R)
        dma_engs = [nc.sync, nc.scalar, nc.gpsimd]
        HALF = 1680
        # earlier segments first: stats chunks in the first third of the
        # image unblock while later thirds still gather. Queue weights favor
        # sync + gpsimd (scalar also runs Prelu/epilogue ACT work).
        SEGS = ((0, 1200), (1200, 2400), (2400, GLEN))
        eng_cycle = [nc.sync, nc.gpsimd, nc.scalar, nc.gpsimd, nc.sync]
        nd = 0
        for c0, c1 in SEGS:
            for p in range(P25):
                i, j = p // KS, p % KS
                off = WP * i + j
                r0 = 0 if p == 0 else row_base(p)
                r1 = MP if p == P25 - 1 else row_base(p) + 4
                eng_cycle[nd % len(eng_cycle)].dma_start(
                    bc[r0:r1, c0:c1],
                    ag[r0:r1, off + c0 : off + c1],
                )
                nd += 1

        # stats: out[pos, stat] = sum_rows bc[row, pos] * smat[row, stat]
        ps_stats = pss_pool.tile([S_CHUNK, N_SCHUNK, 8], F32)
        for s in range(N_SCHUNK):
            off = S_OFFS[s]
            nc.tensor.matmul(
                ps_stats[:, s, :],
                bc[:, off : off + S_CHUNK],
                smat_sb[:],
                start=True,
                stop=True,
            )

        # epilogue: out = csum * exp(-0.5 * sqrt(ysum^2 + xsum^2) / asum)
        # squares on DVE (via an SBUF copy) so ACT only needs Sqrt + Exp
        rinv = tpool.tile([S_CHUNK, N_SCHUNK, NCLS], F32)
        yx = tpool.tile([S_CHUNK, N_SCHUNK, 4], F32)
        yx2 = tpool.tile([S_CHUNK, N_SCHUNK, 4], F32)
        ssum = tpool.tile([S_CHUNK, N_SCHUNK, NCLS], F32)
        srt = tpool.tile([S_CHUNK, N_SCHUNK, NCLS], F32)
        drift = tpool.tile([S_CHUNK, N_SCHUNK, NCLS], F32)
        expd = tpool.tile([S_CHUNK, N_SCHUNK, NCLS], F32)
        outv = tpool.tile([S_CHUNK, N_SCHUNK, NCLS], F32)
        nc.vector.reciprocal(rinv[:], ps_stats[:, :, 0:2])
        nc.vector.tensor_copy(yx[:], ps_stats[:, :, 2:6])
        nc.vector.tensor_mul(yx2[:], yx[:], yx[:])
        nc.vector.tensor_add(ssum[:], yx2[:, :, 0:2], yx2[:, :, 2:4])
        nc.scalar.activation(srt[:], ssum[:], AF.Sqrt)
        nc.vector.tensor_mul(drift[:], srt[:], rinv[:])
        nc.scalar.activation(expd[:], drift[:], AF.Exp, scale=-0.5)
        nc.vector.tensor_mul(outv[:], ps_stats[:, :, 6:8], expd[:])

        # output: pixel (h=2c+d, w) lives at outv[60d+w, c] (chunks 0..26)
        # and outv[60d+4+w, 27] (shifted last chunk); halo partitions skipped
        out_dv = out_d[:].rearrange("(c d w) k -> d w c k", d=2, w=W)
        nlast = N_SCHUNK - 1
        for d in range(2):
            nc.sync.dma_start(
                out_dv[d, :, 0:nlast, :], outv[60 * d : 60 * d + W, 0:nlast, :]
            )
            nc.gpsimd.dma_start(
                out_dv[d, :, nlast, :],
                outv[60 * d + S_SHIFT_LAST : 60 * d + S_SHIFT_LAST + W, nlast, :],
            )


def build_program():
    nc = bacc.Bacc("TRN2", target_bir_lowering=False, debug=False)
    x_d = nc.dram_tensor("x", [C, NPIX], F32R, kind="ExternalInput").ap()
    weff_d = nc.dram_tensor("weff", [C, MP], F32R, kind="ExternalInput").ap()
    actp_d = nc.dram_tensor("actp", [MP, 2], F32, kind="ExternalInput").ap()
    smat_d = nc.dram_tensor("smat", [MP, 8], F32R, kind="ExternalInput").ap()
    out_d = nc.dram_tensor("out", [NOUT, NCLS], F32, kind="ExternalOutput").ap()
    with tile.TileContext(nc) as tc:
        kernel_body(tc, x_d, weff_d, actp_d, smat_d, out_d)
    nc.compile()
    return nc


def host_params(Wt, bias, Wlin, blin):
    """Fold conv weights + linear projection into device params."""
    Wt = np.asarray(Wt, np.float32)
    bias = np.asarray(bias, np.float32)
    Wlin = np.asarray(Wlin, np.float32)
    blin = np.asarray(blin, np.float32)
    O = Wt.shape[0]
    Wp = Wt.reshape(O, C, P25)                        # (O, C, P)
    Weff = np.einsum("ko,ocp->kcp", Wlin, Wp)         # (2, C, P)
    beff2 = (Wlin @ bias + blin).astype(np.float32)   # (2,)
    offs = np.arange(-PAD, PAD + 1, dtype=np.float32)

    wext = np.zeros((MP, C), np.float32)
    actp = np.zeros((MP, 2), np.float32)   # col0 = bias, col1 = prelu alpha
    actp[:, 1] = 1.0
    smat = np.zeros((MP, 8), np.float32)
    for p in range(P25):
        i, j = p // KS, p % KS
        for u in range(4):
            k = u % 2
            r = row_base(p) + u
            wext[r] = Weff[k, :, p]
            actp[r, 0] = beff2[k]
            actp[r, 1] = 1.0 if u < 2 else -1.0
            rb = r + BCSH                     # bc row (shifted vs ag row)
            if u < 2:
                smat[rb, 6 + k] = 1.0          # csum (G' rows)
            else:
                smat[rb, 0 + k] = 1.0          # asum
                smat[rb, 2 + k] = offs[i]      # ysum (yofs)
                smat[rb, 4 + k] = offs[j]      # xsum (xofs)
    weff = np.ascontiguousarray(wext.T)
    return weff, actp, smat


_nc_cache = None
last_results = None  # BassKernelResults of the most recent run (for profiling)


def kernel(x, Wt, bias, Wlin, blin):
    global _nc_cache, last_results
    x = np.asarray(x, np.float32)
    xpad = np.ascontiguousarray(
        np.pad(x, ((0, 0), (0, 0), (PAD, PAD), (PAD, PAD))).reshape(N_CORES, C, NPIX)
    )
    weff, actp, smat = host_params(Wt, bias, Wlin, blin)
    if _nc_cache is None:
        _nc_cache = build_program()
    in_maps = [
        {
            "x": xpad[n],
            "weff": weff,
            "actp": actp,
            "smat": smat,
        }
        for n in range(N_CORES)
    ]
    res = run_bass_kernel_spmd(_nc_cache, in_maps, list(range(N_CORES)))
    last_results = res
    out = np.stack(
        [res.results[n]["out"].reshape(H, W, NCLS) for n in range(N_CORES)]
    )
    return out


# revision 21
# speedup vs baseline: 1.0417x; 1.0207x over previous
"""Trainium2 Bass kernel for CalibConv (5x5 conv -> linear -> drift modulation).

Math: the reference computes, per kernel position p=(i,j) and class k:
    cmap[n,h,w,p,k] = sum_c x_pad[n,c,h+i,w+j] * Weff[k,c,p] + beff[k]
where Weff[k,c,p] = sum_o Wlin[k,o]*Wt[o,c,i,j] and beff = Wlin@bias + blin
(the O=64 conv channels are linearly projected to 2 classes, so they fold
into the weights on the host). Then per output pixel:
    asum = sum_p |cmap|, ysum = sum_p yofs[p]*|cmap|, xsum likewise,
    csum = sum_p cmap,  out = csum * exp(-0.5*sqrt(ysum^2+xsum^2)/asum)

Device strategy (per core; data-parallel over batch N=8 across 8 cores):
  1. x arrives host-padded [C, 60*60]; DMA to SBUF (queues interleaved)
  2. one fp32r matmul pass per column chunk: weff has FOUR weight columns
     per kernel position p, at rows 4p+2+u (u=0,1: G' for class k=u;
     u=2,3: the same weights again for the |.| branch) -> psum [128, F].
     A single ACT Prelu per chunk applies bias and computes BOTH branches:
     alpha=+1 rows pass through (G'+beff), alpha=-1 rows give |G'+beff|.
  3. gather: the window shift for p=(i,j) in 60-stride padded layout is a
     pure flat offset 60*i+j, so each gather is one contiguous 4-partition
     SBUF->SBUF copy ag[4p+2:4p+6, off:off+L] -> bc[4p+2:4p+6, 0:L].
     Rows straddle two SBUF port groups (base 4p+2) for 2x DMA bandwidth.
  4. stats: per 2-padded-row chunk (120 positions x 28), matmul with
     stationary bc[:, off:off+120] [128 x 120] and moving smat [128, 8]
     -> PSUM [120 positions, 8 stats]; horizontal-halo rows are junk,
     skipped by the output DMA.
  5. pixel-major epilogue; 4 small output DMAs [3136, 2]
"""

import numpy as np

import concourse.bacc as bacc
import concourse.mybir as mybir
from concourse import tile
from concourse.bass_utils import run_bass_kernel_spmd

N_CORES = 8
C, H, W = 128, 56, 56
KS, PAD = 5, 2
HP, WP = H + 2 * PAD, W + 2 * PAD  # 60, 60
NPIX = HP * WP                      # 3600
NOUT = H * W                        # 3136
P25 = KS * KS                       # 25
NCLS = 2
MP = 128                            # row space: 4p+2+u (rows 0,1,102.. junk)
GLEN = (H - 1) * WP + W             # 3356: gather copy length per row

G_CHUNK = 450
N_GCHUNK = NPIX // G_CHUNK          # 8
S_CHUNK = 2 * WP                    # 120: stats stationary = 2 padded rows
N_SCHUNK = H // 2                   # 28 chunks (h rows 2c, 2c+1)
S_OFFS = [S_CHUNK * c for c in range(N_SCHUNK - 1)] + [GLEN - S_CHUNK]
S_SHIFT_LAST = S_CHUNK * (N_SCHUNK - 1) - (GLEN - S_CHUNK)  # 4

F32 = mybir.dt.float32
F32R = mybir.dt.float32r
AF = mybir.ActivationFunctionType


def row_base(p):
    return 4 * p + 2


BCSH = 0  # bc rows aligned with ag rows


def kernel_body(tc, x_d, weff_d, actp_d, smat_d, out_d):
    nc = tc.nc
    with (
        tc.tile_pool(name="const", bufs=1) as cpool,
        tc.tile_pool(name="big", bufs=1) as bpool,
        tc.tile_pool(name="psg", bufs=4, space="PSUM") as psg_pool,
        tc.tile_pool(name="pss", bufs=1, space="PSUM") as pss_pool,
        tc.tile_pool(name="tmp", bufs=1) as tpool,
    ):
        # warm the ACT Prelu table before the G phase
        warm_in = cpool.tile([1, 1], F32)
        warm_out = cpool.tile([1, 1], F32)
        nc.vector.memset(warm_in[:], 1.0)
        nc.scalar.activation(warm_out[:], warm_in[:], AF.Prelu, alpha=0.5)

        weff_sb = cpool.tile([C, MP], F32R)
        actp_sb = cpool.tile([MP, 2], F32)
        smat_sb = cpool.tile([MP, 8], F32R)
        nc.gpsimd.dma_start(weff_sb[:], weff_d[:])
        nc.gpsimd.dma_start(actp_sb[:], actp_d[:])
        nc.gpsimd.dma_start(smat_sb[:], smat_d[:])

        # input arrives pre-padded [C, 60*60]; interleave the two HWDGE queues
        xp = bpool.tile([C, NPIX], F32R)
        for b in range(N_GCHUNK):
            eng = nc.sync if b % 2 == 0 else nc.scalar
            eng.dma_start(
                xp[:, G_CHUNK * b : G_CHUNK * (b + 1)],
                x_d[:, G_CHUNK * b : G_CHUNK * (b + 1)],
            )

        # G phase: matmul + one Prelu per chunk writes all 100 live rows
        ag = bpool.tile([MP, NPIX], F32R)
        for g in range(N_GCHUNK):
            sl = slice(G_CHUNK * g, G_CHUNK * (g + 1))
            ps = psg_pool.tile([MP, G_CHUNK], F32, tag="psg")
            nc.tensor.matmul(ps[:], weff_sb[:], xp[:, sl], start=True, stop=True)
            nc.scalar.activation(
                ag[:, sl], ps[:], AF.Prelu,
                bias=actp_sb[:, 0:1], alpha=actp_sb[:, 1:2],
            )

        # ACT tables for the epilogue load now (engine order: after the G
        # Prelus, so the loads hide behind the gather phase)
        nc.scalar.activation(warm_out[:], warm_in[:], AF.Exp)
        nc.scalar.activation(warm_out[:], warm_in[:], AF.Sqrt)

        # gather: contiguous flat-offset copies (rows 4p+2..4p+5); first and
        # last extend to cover the junk rows so bc is fully initialized
        bc = bpool.tile([MP, NPIX], F32R)
        dma_engs = [nc.sync, nc.scalar, nc.gpsimd]
        HALF = 1680
        # earlier segments first: stats chunks in the first third of the
        # image unblock while later thirds still gather. Queue weights favor
        # sync + gpsimd (scalar also runs Prelu/epilogue ACT work).
        SEGS = ((0, HALF), (HALF, GLEN))
        eng_cycle = [nc.sync, nc.gpsimd, nc.scalar, nc.gpsimd, nc.sync]
        nd = 0
        for c0, c1 in SEGS:
            for p in range(P25):
                i, j = p // KS, p % KS
                off = WP * i + j
                r0 = 0 if p == 0 else row_base(p)
                r1 = MP if p == P25 - 1 else row_base(p) + 4
                eng_cycle[nd % len(eng_cycle)].dma_start(
                    bc[r0:r1, c0:c1],
                    ag[r0:r1, off + c0 : off + c1],
                )
                nd += 1

        # stats: out[pos, stat] = sum_rows bc[row, pos] * smat[row, stat]
        ps_stats = pss_pool.tile([S_CHUNK, N_SCHUNK, 8], F32)
        for s in range(N_SCHUNK):
            off = S_OFFS[s]
            nc.tensor.matmul(
                ps_stats[:, s, :],
                bc[:, off : off + S_CHUNK],
                smat_sb[:],
                start=True,
                stop=True,
            )

        # epilogue: out = csum * exp(-0.5 * sqrt(ysum^2 + xsum^2) / asum)
        # squares on DVE (via an SBUF copy) so ACT only needs Sqrt + Exp.
        # Two chunk groups: group 0's output DMA overlaps group 1's compute.
        rinv = tpool.tile([S_CHUNK, N_SCHUNK, NCLS], F32)
        yx = tpool.tile([S_CHUNK, N_SCHUNK, 4], F32)
        yx2 = tpool.tile([S_CHUNK, N_SCHUNK, 4], F32)
        ssum = tpool.tile([S_CHUNK, N_SCHUNK, NCLS], F32)
        srt = tpool.tile([S_CHUNK, N_SCHUNK, NCLS], F32)
        drift = tpool.tile([S_CHUNK, N_SCHUNK, NCLS], F32)
        expd = tpool.tile([S_CHUNK, N_SCHUNK, NCLS], F32)
        outv = tpool.tile([S_CHUNK, N_SCHUNK, NCLS], F32)
        out_dv = out_d[:].rearrange("(c d w) k -> d w c k", d=2, w=W)
        nlast = N_SCHUNK - 1
        GRP = ((0, 14), (14, N_SCHUNK))
        for g0, g1 in GRP:
            s_ = slice(g0, g1)
            nc.vector.reciprocal(rinv[:, s_, :], ps_stats[:, s_, 0:2])
            nc.vector.tensor_copy(yx[:, s_, :], ps_stats[:, s_, 2:6])
            nc.vector.tensor_mul(yx2[:, s_, :], yx[:, s_, :], yx[:, s_, :])
            nc.vector.tensor_add(
                ssum[:, s_, :], yx2[:, s_, 0:2], yx2[:, s_, 2:4]
            )
            nc.scalar.activation(srt[:, s_, :], ssum[:, s_, :], AF.Sqrt)
            nc.vector.tensor_mul(drift[:, s_, :], srt[:, s_, :], rinv[:, s_, :])
            nc.scalar.activation(expd[:, s_, :], drift[:, s_, :], AF.Exp, scale=-0.5)
            nc.vector.tensor_mul(
                outv[:, s_, :], ps_stats[:, s_, 6:8], expd[:, s_, :]
            )
            ce = min(g1, nlast)
            for d in range(2):
                nc.sync.dma_start(
                    out_dv[d, :, g0:ce, :], outv[60 * d : 60 * d + W, g0:ce, :]
                )
            if g1 == N_SCHUNK:
                for d in range(2):
                    nc.gpsimd.dma_start(
                        out_dv[d, :, nlast, :],
                        outv[
                            60 * d + S_SHIFT_LAST : 60 * d + S_SHIFT_LAST + W,
                            nlast,
                            :,
                        ],
                    )


def build_program():
    nc = bacc.Bacc("TRN2", target_bir_lowering=False, debug=False)
    x_d = nc.dram_tensor("x", [C, NPIX], F32R, kind="ExternalInput").ap()
    weff_d = nc.dram_tensor("weff", [C, MP], F32R, kind="ExternalInput").ap()
    actp_d = nc.dram_tensor("actp", [MP, 2], F32, kind="ExternalInput").ap()
    smat_d = nc.dram_tensor("smat", [MP, 8], F32R, kind="ExternalInput").ap()
    out_d = nc.dram_tensor("out", [NOUT, NCLS], F32, kind="ExternalOutput").ap()
    with tile.TileContext(nc) as tc:
        kernel_body(tc, x_d, weff_d, actp_d, smat_d, out_d)
    nc.compile()
    return nc


def host_params(Wt, bias, Wlin, blin):
    """Fold conv weights + linear projection into device params."""
    Wt = np.asarray(Wt, np.float32)
    bias = np.asarray(bias, np.float32)
    Wlin = np.asarray(Wlin, np.float32)
    blin = np.asarray(blin, np.float32)
    O = Wt.shape[0]
    Wp = Wt.reshape(O, C, P25)                        # (O, C, P)
    Weff = np.einsum("ko,ocp->kcp", Wlin, Wp)         # (2, C, P)
    beff2 = (Wlin @ bias + blin).astype(np.float32)   # (2,)
    offs = np.arange(-PAD, PAD + 1, dtype=np.float32)

    wext = np.zeros((MP, C), np.float32)
    actp = np.zeros((MP, 2), np.float32)   # col0 = bias, col1 = prelu alpha
    actp[:, 1] = 1.0
    smat = np.zeros((MP, 8), np.float32)
    for p in range(P25):
        i, j = p // KS, p % KS
        for u in range(4):
            k = u % 2
            r = row_base(p) + u
            wext[r] = Weff[k, :, p]
            actp[r, 0] = beff2[k]
            actp[r, 1] = 1.0 if u < 2 else -1.0
            rb = r + BCSH                     # bc row (shifted vs ag row)
            if u < 2:
                smat[rb, 6 + k] = 1.0          # csum (G' rows)
            else:
                smat[rb, 0 + k] = 1.0          # asum
                smat[rb, 2 + k] = offs[i]      # ysum (yofs)
                smat[rb, 4 + k] = offs[j]      # xsum (xofs)
    weff = np.ascontiguousarray(wext.T)
    return weff, actp, smat


_nc_cache = None
last_results = None  # BassKernelResults of the most recent run (for profiling)


def kernel(x, Wt, bias, Wlin, blin):
    global _nc_cache, last_results
    x = np.asarray(x, np.float32)
    xpad = np.ascontiguousarray(
        np.pad(x, ((0, 0), (0, 0), (PAD, PAD), (PAD, PAD))).reshape(N_CORES, C, NPIX)
    )
    weff, actp, smat = host_params(Wt, bias, Wlin, blin)
    if _nc_cache is None:
        _nc_cache = build_program()
    in_maps = [
        {
            "x": xpad[n],
            "weff": weff,
            "actp": actp,
            "smat": smat,
        }
        for n in range(N_CORES)
    ]
    res = run_bass_kernel_spmd(_nc_cache, in_maps, list(range(N_CORES)))
    last_results = res
    out = np.stack(
        [res.results[n]["out"].reshape(H, W, NCLS) for n in range(N_CORES)]
    )
    return out


# revision 22
# speedup vs baseline: 1.0631x; 1.0205x over previous
"""Trainium2 Bass kernel for CalibConv (5x5 conv -> linear -> drift modulation).

Math: the reference computes, per kernel position p=(i,j) and class k:
    cmap[n,h,w,p,k] = sum_c x_pad[n,c,h+i,w+j] * Weff[k,c,p] + beff[k]
where Weff[k,c,p] = sum_o Wlin[k,o]*Wt[o,c,i,j] and beff = Wlin@bias + blin
(the O=64 conv channels are linearly projected to 2 classes, so they fold
into the weights on the host). Then per output pixel:
    asum = sum_p |cmap|, ysum = sum_p yofs[p]*|cmap|, xsum likewise,
    csum = sum_p cmap,  out = csum * exp(-0.5*sqrt(ysum^2+xsum^2)/asum)

Device strategy (per core; data-parallel over batch N=8 across 8 cores):
  1. x arrives host-padded [C, 60*60]; DMA to SBUF (queues interleaved)
  2. one fp32r matmul pass per column chunk: weff has FOUR weight columns
     per kernel position p, at rows 4p+2+u (u=0,1: G' for class k=u;
     u=2,3: the same weights again for the |.| branch) -> psum [128, F].
     A single ACT Prelu per chunk applies bias and computes BOTH branches:
     alpha=+1 rows pass through (G'+beff), alpha=-1 rows give |G'+beff|.
  3. gather: the window shift for p=(i,j) in 60-stride padded layout is a
     pure flat offset 60*i+j, so each gather is one contiguous 4-partition
     SBUF->SBUF copy ag[4p+2:4p+6, off:off+L] -> bc[4p+2:4p+6, 0:L].
     Rows straddle two SBUF port groups (base 4p+2) for 2x DMA bandwidth.
  4. stats: per 2-padded-row chunk (120 positions x 28), matmul with
     stationary bc[:, off:off+120] [128 x 120] and moving smat [128, 8]
     -> PSUM [120 positions, 8 stats]; horizontal-halo rows are junk,
     skipped by the output DMA.
  5. pixel-major epilogue; 4 small output DMAs [3136, 2]
"""

import numpy as np

import concourse.bacc as bacc
import concourse.mybir as mybir
from concourse import tile
from concourse.bass_utils import run_bass_kernel_spmd

N_CORES = 8
C, H, W = 128, 56, 56
KS, PAD = 5, 2
HP, WP = H + 2 * PAD, W + 2 * PAD  # 60, 60
NPIX = HP * WP                      # 3600
NOUT = H * W                        # 3136
P25 = KS * KS                       # 25
NCLS = 2
MP = 128                            # row space: 4p+2+u (rows 0,1,102.. junk)
GLEN = (H - 1) * WP + W             # 3356: gather copy length per row

G_CHUNK = 450
N_GCHUNK = NPIX // G_CHUNK          # 8
S_CHUNK = 2 * WP                    # 120: stats stationary = 2 padded rows
N_SCHUNK = H // 2                   # 28 chunks (h rows 2c, 2c+1)
S_OFFS = [S_CHUNK * c for c in range(N_SCHUNK - 1)] + [GLEN - S_CHUNK]
S_SHIFT_LAST = S_CHUNK * (N_SCHUNK - 1) - (GLEN - S_CHUNK)  # 4

F32 = mybir.dt.float32
F32R = mybir.dt.float32r
AF = mybir.ActivationFunctionType


def row_base(p):
    return 4 * p + 2


BCSH = 0  # bc rows aligned with ag rows


def kernel_body(tc, x_d, weff_d, actp_d, smat_d, out_d):
    nc = tc.nc
    with (
        tc.tile_pool(name="const", bufs=1) as cpool,
        tc.tile_pool(name="big", bufs=1) as bpool,
        tc.tile_pool(name="psg", bufs=6, space="PSUM") as psg_pool,
        tc.tile_pool(name="pss", bufs=1, space="PSUM") as pss_pool,
        tc.tile_pool(name="tmp", bufs=1) as tpool,
    ):
        # warm the ACT Prelu table before the G phase
        warm_in = cpool.tile([1, 1], F32)
        warm_out = cpool.tile([1, 1], F32)
        nc.vector.memset(warm_in[:], 1.0)
        nc.scalar.activation(warm_out[:], warm_in[:], AF.Prelu, alpha=0.5)

        weff_sb = cpool.tile([C, MP], F32R)
        actp_sb = cpool.tile([MP, 2], F32)
        smat_sb = cpool.tile([MP, 8], F32R)
        nc.gpsimd.dma_start(weff_sb[:], weff_d[:])
        nc.gpsimd.dma_start(actp_sb[:], actp_d[:])
        nc.gpsimd.dma_start(smat_sb[:], smat_d[:])

        # input arrives pre-padded [C, 60*60]; interleave the two HWDGE queues
        xp = bpool.tile([C, NPIX], F32R)
        for b in range(N_GCHUNK):
            eng = nc.sync if b % 2 == 0 else nc.scalar
            eng.dma_start(
                xp[:, G_CHUNK * b : G_CHUNK * (b + 1)],
                x_d[:, G_CHUNK * b : G_CHUNK * (b + 1)],
            )

        # G phase: matmul + one Prelu per chunk writes all 100 live rows
        ag = bpool.tile([MP, NPIX], F32R)
        for g in range(N_GCHUNK):
            sl = slice(G_CHUNK * g, G_CHUNK * (g + 1))
            ps = psg_pool.tile([MP, G_CHUNK], F32, tag="psg")
            nc.tensor.matmul(ps[:], weff_sb[:], xp[:, sl], start=True, stop=True)
            nc.scalar.activation(
                ag[:, sl], ps[:], AF.Prelu,
                bias=actp_sb[:, 0:1], alpha=actp_sb[:, 1:2],
            )

        # ACT tables for the epilogue load now (engine order: after the G
        # Prelus, so the loads hide behind the gather phase)
        nc.scalar.activation(warm_out[:], warm_in[:], AF.Exp)
        nc.scalar.activation(warm_out[:], warm_in[:], AF.Sqrt)

        # gather: contiguous flat-offset copies (rows 4p+2..4p+5); first and
        # last extend to cover the junk rows so bc is fully initialized
        bc = bpool.tile([MP, NPIX], F32R)
        dma_engs = [nc.sync, nc.scalar, nc.gpsimd]
        HALF = 1680
        # earlier segments first: stats chunks in the first third of the
        # image unblock while later thirds still gather. Queue weights favor
        # sync + gpsimd (scalar also runs Prelu/epilogue ACT work).
        SEGS = ((0, HALF), (HALF, GLEN))
        eng_cycle = [nc.sync, nc.gpsimd, nc.scalar, nc.gpsimd, nc.sync]
        nd = 0
        for c0, c1 in SEGS:
            for p in range(P25):
                i, j = p // KS, p % KS
                off = WP * i + j
                r0 = 0 if p == 0 else row_base(p)
                r1 = MP if p == P25 - 1 else row_base(p) + 4
                eng_cycle[nd % len(eng_cycle)].dma_start(
                    bc[r0:r1, c0:c1],
                    ag[r0:r1, off + c0 : off + c1],
                )
                nd += 1

        # stats: out[pos, stat] = sum_rows bc[row, pos] * smat[row, stat]
        ps_stats = pss_pool.tile([S_CHUNK, N_SCHUNK, 8], F32)
        for s in range(N_SCHUNK):
            off = S_OFFS[s]
            nc.tensor.matmul(
                ps_stats[:, s, :],
                bc[:, off : off + S_CHUNK],
                smat_sb[:],
                start=True,
                stop=True,
            )

        # epilogue: out = csum * exp(-0.5 * sqrt(ysum^2 + xsum^2) / asum)
        # squares on DVE (via an SBUF copy) so ACT only needs Sqrt + Exp.
        # Two chunk groups: group 0's output DMA overlaps group 1's compute.
        rinv = tpool.tile([S_CHUNK, N_SCHUNK, NCLS], F32)
        yx = tpool.tile([S_CHUNK, N_SCHUNK, 4], F32)
        yx2 = tpool.tile([S_CHUNK, N_SCHUNK, 4], F32)
        ssum = tpool.tile([S_CHUNK, N_SCHUNK, NCLS], F32)
        srt = tpool.tile([S_CHUNK, N_SCHUNK, NCLS], F32)
        drift = tpool.tile([S_CHUNK, N_SCHUNK, NCLS], F32)
        expd = tpool.tile([S_CHUNK, N_SCHUNK, NCLS], F32)
        outv = tpool.tile([S_CHUNK, N_SCHUNK, NCLS], F32)
        out_dv = out_d[:].rearrange("(c d w) k -> d w c k", d=2, w=W)
        nlast = N_SCHUNK - 1
        GRP = ((0, 14), (14, N_SCHUNK))
        # pass 1 per group: everything through sqrt (ACT stays on Sqrt)
        for g0, g1 in GRP:
            s_ = slice(g0, g1)
            nc.vector.reciprocal(rinv[:, s_, :], ps_stats[:, s_, 0:2])
            nc.vector.tensor_copy(yx[:, s_, :], ps_stats[:, s_, 2:6])
            nc.vector.tensor_mul(yx2[:, s_, :], yx[:, s_, :], yx[:, s_, :])
            nc.vector.tensor_add(
                ssum[:, s_, :], yx2[:, s_, 0:2], yx2[:, s_, 2:4]
            )
            nc.scalar.activation(srt[:, s_, :], ssum[:, s_, :], AF.Sqrt)
        # pass 2 per group: exp, final mul, store (one Exp table load)
        for g0, g1 in GRP:
            s_ = slice(g0, g1)
            nc.vector.tensor_mul(drift[:, s_, :], srt[:, s_, :], rinv[:, s_, :])
            nc.scalar.activation(expd[:, s_, :], drift[:, s_, :], AF.Exp, scale=-0.5)
            nc.vector.tensor_mul(
                outv[:, s_, :], ps_stats[:, s_, 6:8], expd[:, s_, :]
            )
            ce = min(g1, nlast)
            for d in range(2):
                nc.sync.dma_start(
                    out_dv[d, :, g0:ce, :], outv[60 * d : 60 * d + W, g0:ce, :]
                )
            if g1 == N_SCHUNK:
                for d in range(2):
                    nc.gpsimd.dma_start(
                        out_dv[d, :, nlast, :],
                        outv[
                            60 * d + S_SHIFT_LAST : 60 * d + S_SHIFT_LAST + W,
                            nlast,
                            :,
                        ],
                    )


def build_program():
    nc = bacc.Bacc("TRN2", target_bir_lowering=False, debug=False)
    x_d = nc.dram_tensor("x", [C, NPIX], F32R, kind="ExternalInput").ap()
    weff_d = nc.dram_tensor("weff", [C, MP], F32R, kind="ExternalInput").ap()
    actp_d = nc.dram_tensor("actp", [MP, 2], F32, kind="ExternalInput").ap()
    smat_d = nc.dram_tensor("smat", [MP, 8], F32R, kind="ExternalInput").ap()
    out_d = nc.dram_tensor("out", [NOUT, NCLS], F32, kind="ExternalOutput").ap()
    with tile.TileContext(nc) as tc:
        kernel_body(tc, x_d, weff_d, actp_d, smat_d, out_d)
    nc.compile()
    return nc


def host_params(Wt, bias, Wlin, blin):
    """Fold conv weights + linear projection into device params."""
    Wt = np.asarray(Wt, np.float32)
    bias = np.asarray(bias, np.float32)
    Wlin = np.asarray(Wlin, np.float32)
    blin = np.asarray(blin, np.float32)
    O = Wt.shape[0]
    Wp = Wt.reshape(O, C, P25)                        # (O, C, P)
    Weff = np.einsum("ko,ocp->kcp", Wlin, Wp)         # (2, C, P)
    beff2 = (Wlin @ bias + blin).astype(np.float32)   # (2,)
    offs = np.arange(-PAD, PAD + 1, dtype=np.float32)

    wext = np.zeros((MP, C), np.float32)
    actp = np.zeros((MP, 2), np.float32)   # col0 = bias, col1 = prelu alpha
    actp[:, 1] = 1.0
    smat = np.zeros((MP, 8), np.float32)
    for p in range(P25):
        i, j = p // KS, p % KS
        for u in range(4):
            k = u % 2
            r = row_base(p) + u
            wext[r] = Weff[k, :, p]
            actp[r, 0] = beff2[k]
            actp[r, 1] = 1.0 if u < 2 else -1.0
            rb = r + BCSH                     # bc row (shifted vs ag row)
            if u < 2:
                smat[rb, 6 + k] = 1.0          # csum (G' rows)
            else:
                smat[rb, 0 + k] = 1.0          # asum
                smat[rb, 2 + k] = offs[i]      # ysum (yofs)
                smat[rb, 4 + k] = offs[j]      # xsum (xofs)
    weff = np.ascontiguousarray(wext.T)
    return weff, actp, smat


_nc_cache = None
last_results = None  # BassKernelResults of the most recent run (for profiling)


def kernel(x, Wt, bias, Wlin, blin):
    global _nc_cache, last_results
    x = np.asarray(x, np.float32)
    xpad = np.ascontiguousarray(
        np.pad(x, ((0, 0), (0, 0), (PAD, PAD), (PAD, PAD))).reshape(N_CORES, C, NPIX)
    )
    weff, actp, smat = host_params(Wt, bias, Wlin, blin)
    if _nc_cache is None:
        _nc_cache = build_program()
    in_maps = [
        {
            "x": xpad[n],
            "weff": weff,
            "actp": actp,
            "smat": smat,
        }
        for n in range(N_CORES)
    ]
    res = run_bass_kernel_spmd(_nc_cache, in_maps, list(range(N_CORES)))
    last_results = res
    out = np.stack(
        [res.results[n]["out"].reshape(H, W, NCLS) for n in range(N_CORES)]
    )
    return out


# revision 23
# speedup vs baseline: 1.1331x; 1.0658x over previous
"""Trainium2 Bass kernel for CalibConv (5x5 conv -> linear -> drift modulation).

Math: the reference computes, per kernel position p=(i,j) and class k:
    cmap[n,h,w,p,k] = sum_c x_pad[n,c,h+i,w+j] * Weff[k,c,p] + beff[k]
where Weff[k,c,p] = sum_o Wlin[k,o]*Wt[o,c,i,j] and beff = Wlin@bias + blin
(the O=64 conv channels are linearly projected to 2 classes, so they fold
into the weights on the host). Then per output pixel:
    asum = sum_p |cmap|, ysum = sum_p yofs[p]*|cmap|, xsum likewise,
    csum = sum_p cmap,  out = csum * exp(-0.5*sqrt(ysum^2+xsum^2)/asum)

Device strategy (per core; data-parallel over batch N=8 across 8 cores):
  1. x arrives host-padded [C, 60*60]; DMA to SBUF (queues interleaved)
  2. one fp32r matmul pass per column chunk: weff has FOUR weight columns
     per kernel position p, at rows 4p+2+u (u=0,1: G' for class k=u;
     u=2,3: the same weights again for the |.| branch) -> psum [128, F].
     A single ACT Prelu per chunk applies bias and computes BOTH branches:
     alpha=+1 rows pass through (G'+beff), alpha=-1 rows give |G'+beff|.
  3. gather: the window shift for p=(i,j) in 60-stride padded layout is a
     pure flat offset 60*i+j, so each gather is one contiguous 4-partition
     SBUF->SBUF copy ag[4p+2:4p+6, off:off+L] -> bc[4p+2:4p+6, 0:L].
     Rows straddle two SBUF port groups (base 4p+2) for 2x DMA bandwidth.
  4. stats: per 2-padded-row chunk (120 positions x 28), matmul with
     stationary bc[:, off:off+120] [128 x 120] and moving smat [128, 8]
     -> PSUM [120 positions, 8 stats]; horizontal-halo rows are junk,
     skipped by the output DMA.
  5. pixel-major epilogue; 4 small output DMAs [3136, 2]
"""

import numpy as np

import concourse.bacc as bacc
import concourse.mybir as mybir
from concourse import tile
from concourse.bass_utils import run_bass_kernel_spmd

N_CORES = 8
C, H, W = 128, 56, 56
KS, PAD = 5, 2
HP, WP = H + 2 * PAD, W + 2 * PAD  # 60, 60
NPIX = HP * WP                      # 3600
NOUT = H * W                        # 3136
P25 = KS * KS                       # 25
NCLS = 2
MP = 128                            # row space: 4p+2+u (rows 0,1,102.. junk)
GLEN = (H - 1) * WP + W             # 3356: gather copy length per row

G_CHUNK = 450
N_GCHUNK = NPIX // G_CHUNK          # 8
S_CHUNK = 2 * WP                    # 120: stats stationary = 2 padded rows
N_SCHUNK = H // 2                   # 28 chunks (h rows 2c, 2c+1)
S_OFFS = [S_CHUNK * c for c in range(N_SCHUNK - 1)] + [GLEN - S_CHUNK]
S_SHIFT_LAST = S_CHUNK * (N_SCHUNK - 1) - (GLEN - S_CHUNK)  # 4

F32 = mybir.dt.float32
F32R = mybir.dt.float32r
BF16 = mybir.dt.bfloat16
AF = mybir.ActivationFunctionType


def row_base(p):
    return 4 * p + 2


BCSH = 0  # bc rows aligned with ag rows


def kernel_body(tc, x_d, weff_d, actp_d, smat_d, out_d):
    nc = tc.nc
    with (
        tc.tile_pool(name="const", bufs=1) as cpool,
        tc.tile_pool(name="big", bufs=1) as bpool,
        tc.tile_pool(name="psg", bufs=6, space="PSUM") as psg_pool,
        tc.tile_pool(name="pss", bufs=1, space="PSUM") as pss_pool,
        tc.tile_pool(name="tmp", bufs=1) as tpool,
    ):
        # warm the ACT Prelu table before the G phase
        warm_in = cpool.tile([1, 1], F32)
        warm_out = cpool.tile([1, 1], F32)
        nc.vector.memset(warm_in[:], 1.0)
        nc.scalar.activation(warm_out[:], warm_in[:], AF.Prelu, alpha=0.5)

        weff_sb = cpool.tile([C, MP], F32R)
        actp_sb = cpool.tile([MP, 2], F32)
        smat_sb = cpool.tile([MP, 8], BF16)
        nc.gpsimd.dma_start(weff_sb[:], weff_d[:])
        nc.gpsimd.dma_start(actp_sb[:], actp_d[:])
        nc.gpsimd.dma_start(smat_sb[:], smat_d[:])

        # input arrives pre-padded [C, 60*60]; interleave the two HWDGE queues
        xp = bpool.tile([C, NPIX], F32R)
        for b in range(N_GCHUNK):
            eng = nc.sync if b % 2 == 0 else nc.scalar
            eng.dma_start(
                xp[:, G_CHUNK * b : G_CHUNK * (b + 1)],
                x_d[:, G_CHUNK * b : G_CHUNK * (b + 1)],
            )

        # G phase: matmul + one Prelu per chunk writes all 100 live rows
        ag = bpool.tile([MP, NPIX], BF16)
        for g in range(N_GCHUNK):
            sl = slice(G_CHUNK * g, G_CHUNK * (g + 1))
            ps = psg_pool.tile([MP, G_CHUNK], F32, tag="psg")
            nc.tensor.matmul(ps[:], weff_sb[:], xp[:, sl], start=True, stop=True)
            nc.scalar.activation(
                ag[:, sl], ps[:], AF.Prelu,
                bias=actp_sb[:, 0:1], alpha=actp_sb[:, 1:2],
            )

        # ACT tables for the epilogue load now (engine order: after the G
        # Prelus, so the loads hide behind the gather phase)
        nc.scalar.activation(warm_out[:], warm_in[:], AF.Exp)
        nc.scalar.activation(warm_out[:], warm_in[:], AF.Sqrt)

        # gather: contiguous flat-offset copies (rows 4p+2..4p+5); first and
        # last extend to cover the junk rows so bc is fully initialized
        bc = bpool.tile([MP, NPIX], BF16)
        dma_engs = [nc.sync, nc.scalar, nc.gpsimd]
        HALF = 1680
        # earlier segments first: stats chunks in the first third of the
        # image unblock while later thirds still gather. Queue weights favor
        # sync + gpsimd (scalar also runs Prelu/epilogue ACT work).
        SEGS = ((0, HALF), (HALF, GLEN))
        eng_cycle = [nc.sync, nc.gpsimd, nc.scalar, nc.gpsimd, nc.sync]
        nd = 0
        for c0, c1 in SEGS:
            for p in range(P25):
                i, j = p // KS, p % KS
                off = WP * i + j
                r0 = 0 if p == 0 else row_base(p)
                r1 = MP if p == P25 - 1 else row_base(p) + 4
                eng_cycle[nd % len(eng_cycle)].dma_start(
                    bc[r0:r1, c0:c1],
                    ag[r0:r1, off + c0 : off + c1],
                )
                nd += 1

        # stats: out[pos, stat] = sum_rows bc[row, pos] * smat[row, stat]
        ps_stats = pss_pool.tile([S_CHUNK, N_SCHUNK, 8], F32)
        for s in range(N_SCHUNK):
            off = S_OFFS[s]
            nc.tensor.matmul(
                ps_stats[:, s, :],
                bc[:, off : off + S_CHUNK],
                smat_sb[:],
                start=True,
                stop=True,
            )

        # epilogue: out = csum * exp(-0.5 * sqrt(ysum^2 + xsum^2) / asum)
        # squares on DVE (via an SBUF copy) so ACT only needs Sqrt + Exp.
        # Two chunk groups: group 0's output DMA overlaps group 1's compute.
        rinv = tpool.tile([S_CHUNK, N_SCHUNK, NCLS], F32)
        yx = tpool.tile([S_CHUNK, N_SCHUNK, 4], F32)
        yx2 = tpool.tile([S_CHUNK, N_SCHUNK, 4], F32)
        ssum = tpool.tile([S_CHUNK, N_SCHUNK, NCLS], F32)
        srt = tpool.tile([S_CHUNK, N_SCHUNK, NCLS], F32)
        drift = tpool.tile([S_CHUNK, N_SCHUNK, NCLS], F32)
        expd = tpool.tile([S_CHUNK, N_SCHUNK, NCLS], F32)
        outv = tpool.tile([S_CHUNK, N_SCHUNK, NCLS], F32)
        out_dv = out_d[:].rearrange("(c d w) k -> d w c k", d=2, w=W)
        nlast = N_SCHUNK - 1
        GRP = ((0, 14), (14, N_SCHUNK))
        # pass 1 per group: everything through sqrt (ACT stays on Sqrt)
        for g0, g1 in GRP:
            s_ = slice(g0, g1)
            nc.vector.reciprocal(rinv[:, s_, :], ps_stats[:, s_, 0:2])
            nc.vector.tensor_copy(yx[:, s_, :], ps_stats[:, s_, 2:6])
            nc.vector.tensor_mul(yx2[:, s_, :], yx[:, s_, :], yx[:, s_, :])
            nc.vector.tensor_add(
                ssum[:, s_, :], yx2[:, s_, 0:2], yx2[:, s_, 2:4]
            )
            nc.scalar.activation(srt[:, s_, :], ssum[:, s_, :], AF.Sqrt)
        # pass 2 per group: exp, final mul, store (one Exp table load)
        for g0, g1 in GRP:
            s_ = slice(g0, g1)
            nc.vector.tensor_mul(drift[:, s_, :], srt[:, s_, :], rinv[:, s_, :])
            nc.scalar.activation(expd[:, s_, :], drift[:, s_, :], AF.Exp, scale=-0.5)
            nc.vector.tensor_mul(
                outv[:, s_, :], ps_stats[:, s_, 6:8], expd[:, s_, :]
            )
            ce = min(g1, nlast)
            for d in range(2):
                nc.sync.dma_start(
                    out_dv[d, :, g0:ce, :], outv[60 * d : 60 * d + W, g0:ce, :]
                )
            if g1 == N_SCHUNK:
                for d in range(2):
                    nc.gpsimd.dma_start(
                        out_dv[d, :, nlast, :],
                        outv[
                            60 * d + S_SHIFT_LAST : 60 * d + S_SHIFT_LAST + W,
                            nlast,
                            :,
                        ],
                    )


def build_program():
    nc = bacc.Bacc("TRN2", target_bir_lowering=False, debug=False)
    x_d = nc.dram_tensor("x", [C, NPIX], F32R, kind="ExternalInput").ap()
    weff_d = nc.dram_tensor("weff", [C, MP], F32R, kind="ExternalInput").ap()
    actp_d = nc.dram_tensor("actp", [MP, 2], F32, kind="ExternalInput").ap()
    smat_d = nc.dram_tensor("smat", [MP, 8], BF16, kind="ExternalInput").ap()
    out_d = nc.dram_tensor("out", [NOUT, NCLS], F32, kind="ExternalOutput").ap()
    with tile.TileContext(nc) as tc:
        kernel_body(tc, x_d, weff_d, actp_d, smat_d, out_d)
    nc.compile()
    return nc


def host_params(Wt, bias, Wlin, blin):
    """Fold conv weights + linear projection into device params."""
    Wt = np.asarray(Wt, np.float32)
    bias = np.asarray(bias, np.float32)
    Wlin = np.asarray(Wlin, np.float32)
    blin = np.asarray(blin, np.float32)
    O = Wt.shape[0]
    Wp = Wt.reshape(O, C, P25)                        # (O, C, P)
    Weff = np.einsum("ko,ocp->kcp", Wlin, Wp)         # (2, C, P)
    beff2 = (Wlin @ bias + blin).astype(np.float32)   # (2,)
    offs = np.arange(-PAD, PAD + 1, dtype=np.float32)

    wext = np.zeros((MP, C), np.float32)
    actp = np.zeros((MP, 2), np.float32)   # col0 = bias, col1 = prelu alpha
    actp[:, 1] = 1.0
    smat = np.zeros((MP, 8), np.float32)
    for p in range(P25):
        i, j = p // KS, p % KS
        for u in range(4):
            k = u % 2
            r = row_base(p) + u
            wext[r] = Weff[k, :, p]
            actp[r, 0] = beff2[k]
            actp[r, 1] = 1.0 if u < 2 else -1.0
            rb = r + BCSH                     # bc row (shifted vs ag row)
            if u < 2:
                smat[rb, 6 + k] = 1.0          # csum (G' rows)
            else:
                smat[rb, 0 + k] = 1.0          # asum
                smat[rb, 2 + k] = offs[i]      # ysum (yofs)
                smat[rb, 4 + k] = offs[j]      # xsum (xofs)
    import ml_dtypes
    weff = np.ascontiguousarray(wext.T)
    smat = smat.astype(ml_dtypes.bfloat16)
    return weff, actp, smat


_nc_cache = None
last_results = None  # BassKernelResults of the most recent run (for profiling)


def kernel(x, Wt, bias, Wlin, blin):
    global _nc_cache, last_results
    x = np.asarray(x, np.float32)
    xpad = np.ascontiguousarray(
        np.pad(x, ((0, 0), (0, 0), (PAD, PAD), (PAD, PAD))).reshape(N_CORES, C, NPIX)
    )
    weff, actp, smat = host_params(Wt, bias, Wlin, blin)
    if _nc_cache is None:
        _nc_cache = build_program()
    in_maps = [
        {
            "x": xpad[n],
            "weff": weff,
            "actp": actp,
            "smat": smat,
        }
        for n in range(N_CORES)
    ]
    res = run_bass_kernel_spmd(_nc_cache, in_maps, list(range(N_CORES)))
    last_results = res
    out = np.stack(
        [res.results[n]["out"].reshape(H, W, NCLS) for n in range(N_CORES)]
    )
    return out
